# revision 15
# baseline (speedup 1.0000x reference)
"""Trainium2 Bass kernel: causal sliding-window attention + output projection.

Reference computation (B=1, H=16, T=2048, D=64, WINDOW=256, DIM=1024):
    att  = softmax(mask(q @ k^T / sqrt(D)))       per head, sliding causal window
    y    = att @ v                                 -> (B, H, T, D)
    out  = y.transpose -> (B, T, H*D) @ W_proj     -> (B, T, DIM)

Sharding over 8 NeuronCores: 2 head-groups (R) x 4 sequence-blocks (S).
Core c = (r, s): heads [8r, 8r+8), queries [512s, 512s+512), key window
[512s-256, 512s+512) (zero-padded below 0).  W_proj row-sharded per head
group; host sums the two partial projections per sequence block
(the "all-reduce after projection" done at gather time).

On-device layout (everything transposed so no on-chip transposes needed):
  scores^T[k, q] = (kT_ext)^T @ qT_ext      (65-row contraction: 64 dims +
                                             bias row giving -1e9 on padded keys)
  P^T = exp(scores * 1/8)                   one ACT op over [128, 1536]
  tri-masks multiply P^T (constant 128x128 triangles; r=1 tiles need none)
  O = [v | ones]^T @ P^T                    -> [128, q]: rows 0:64 = y^T(head),
                                               rows 64:128 = softmax denom (replicated)
  yT = O[0:64] * recip(O[64:128])           per head, written into [128,512] chunks
  out[q, n] = sum_hp yT_hp^T @ W_hp         accumulated in PSUM, DMA'd out
"""

import math
import os
import sys
from contextlib import ExitStack

import numpy as np

for _p in ("/opt/trn_rl_repo",):
    if _p not in sys.path and os.path.isdir(_p):
        sys.path.insert(0, _p)

import ml_dtypes  # noqa: E402

BF16NP = ml_dtypes.bfloat16

B, H, T, D = 1, 16, 2048, 64
DIM = H * D
WINDOW = 256
R, S = 2, 4                 # head groups x sequence blocks
HL = H // R                 # 8 heads per core
QL = T // S                 # 512 queries per core
KW = QL + WINDOW            # 768-key window per core
NKT = KW // 128             # 6 key tiles
NQB = QL // 128             # 4 query blocks
NEG = -1.0e9                # additive bias for padded (out-of-range) keys

# QK matmul pieces: (kt, score_col, q_col, width, start, stop).
# Scores live in one [128, 1536] fp32 PSUM tensor (3 banks of 512 f32 cols).
# Pieces are split so no matmul output crosses a bank boundary; start/stop
# mark the first/last write into each bank.
QK_PIECES = [
    (0, 0, 0, 128, True, False),
    (1, 128, 0, 256, False, False),
    (2, 384, 0, 128, False, True),     # last write to bank 0
    (2, 512, 128, 256, True, False),   # first write to bank 1
    (3, 768, 128, 256, False, True),   # last write to bank 1
    (3, 1024, 384, 128, True, False),  # first write to bank 2
    (4, 1152, 256, 256, False, False),
    (5, 1408, 384, 128, False, True),  # last write to bank 2
]

# Triangular mask multiplies on P^T: (score_col, tri_index) where tri 0 = strict
# lower (p > i), tri 1 = upper-incl-diag (p <= i).  r=1 tiles are fully inside
# the window and need no mask.
MASKS = [
    (0, 0), (256, 0), (640, 0), (1024, 0),
    (384, 1), (768, 1), (1152, 1), (1408, 1),
]

# AV matmuls: (kt, score_col, width, out_q_col).  Pieces are split so each
# matmul's output region is uniformly "fresh" or uniformly "accumulating"
# (CoreSim asserts this; HW per-byte has_written would allow either).
AV_TABLE = [
    (0, 0, 128, 0),
    (1, 128, 128, 0),
    (1, 256, 128, 128),
    (2, 384, 256, 0),
    (2, 640, 128, 256),
    (3, 768, 256, 128),
    (3, 1024, 128, 384),
    (4, 1152, 256, 256),
    (5, 1408, 128, 384),
]


def _emit(tc, qT_d, kT_d, vE_d, Wc_d, tri_d, out_d, taps=None):
    import concourse.mybir as mybir

    nc = tc.nc
    BF16 = mybir.dt.bfloat16
    F32 = mybir.dt.float32
    Exp = mybir.ActivationFunctionType.Exp

    with ExitStack() as ctx:
        const = ctx.enter_context(tc.tile_pool(name="const", bufs=1))
        sc_pool = ctx.enter_context(tc.tile_pool(name="sc", bufs=1, space="PSUM"))
        ot_pool = ctx.enter_context(tc.tile_pool(name="ot", bufs=2, space="PSUM"))
        proj_pool = ctx.enter_context(tc.tile_pool(name="proj", bufs=3, space="PSUM"))
        pt_pool = ctx.enter_context(tc.tile_pool(name="pt", bufs=2))
        rc_pool = ctx.enter_context(tc.tile_pool(name="rc", bufs=2))
        yt_pool = ctx.enter_context(tc.tile_pool(name="yt", bufs=1))
        ost_pool = ctx.enter_context(tc.tile_pool(name="ost", bufs=2))

        # ---- load all inputs into SBUF up front (they fit easily) ----
        q_t = []
        k_t = []
        v_t = []
        for h in range(HL):
            qt = const.tile([65, QL], BF16, tag=f"q{h}")
            nc.sync.dma_start(qt[:], qT_d[h])
            q_t.append(qt)
            kt_ = const.tile([65, KW], BF16, tag=f"k{h}")
            nc.sync.dma_start(kt_[:], kT_d[h])
            k_t.append(kt_)
            vt = const.tile([128, NKT * 128], BF16, tag=f"v{h}")
            nc.sync.dma_start(vt[:], vE_d[h])
            v_t.append(vt)
        w_t = []
        for hp in range(4):
            wt = const.tile([128, DIM], BF16, tag=f"w{hp}")
            nc.sync.dma_start(wt[:], Wc_d[hp])
            w_t.append(wt)
        tri_t = const.tile([128, 256], BF16, tag="tri")
        nc.sync.dma_start(tri_t[:], tri_d.transpose([1, 0, 2]))

        yt_t = [
            yt_pool.tile([128, QL], BF16, tag=f"yt{hp}", name=f"yt{hp}")
            for hp in range(4)
        ]

        # ---- attention per head ----
        for h in range(HL):
            sc = sc_pool.tile([128, 1536], F32, tag="sc")
            for (kt, so, qo, w, st, sp) in QK_PIECES:
                nc.tensor.matmul(
                    sc[:, so:so + w],
                    lhsT=k_t[h][:, kt * 128:(kt + 1) * 128],
                    rhs=q_t[h][:, qo:qo + w],
                    start=st, stop=sp,
                )
            pt = pt_pool.tile([128, 1536], BF16, tag="pt")
            nc.scalar.activation(pt[:], sc[:], Exp, scale=1.0 / math.sqrt(D))
            for (off, which) in MASKS:
                nc.vector.tensor_mul(
                    pt[:, off:off + 128],
                    pt[:, off:off + 128],
                    tri_t[:, which * 128:(which + 1) * 128],
                )
            ot = ot_pool.tile([128, QL], F32, tag="ot")
            for i, (kt, po, w, oq) in enumerate(AV_TABLE):
                nc.tensor.matmul(
                    ot[:, oq:oq + w],
                    lhsT=v_t[h][:, kt * 128:(kt + 1) * 128],
                    rhs=pt[:, po:po + w],
                    start=(i == 0), stop=(i == len(AV_TABLE) - 1),
                )
            # vE packs [ones | v] per key tile, so ot rows 0:64 hold the
            # softmax denominator (replicated) and rows 64:128 hold y^T.
            # Custom DVE ops only work at base partition 0 on HW; this
            # layout keeps the reciprocal there.
            rc = rc_pool.tile([64, QL], F32, tag="rc")
            nc.vector.reciprocal_approx_fast(out=rc[:], in_=ot[0:64, :])
            hp, odd = h // 2, h % 2
            nc.vector.tensor_mul(
                yt_t[hp][odd * 64:(odd + 1) * 64, :], ot[64:128, :], rc[:]
            )
            if taps is not None and h == 0:
                nc.sync.dma_start(taps["pt0"], pt[:])
                nc.sync.dma_start(taps["rc0"], rc[:])
                ots = ost_pool.tile([128, QL], F32, tag="ots", name="ots")
                nc.vector.tensor_copy(ots[:], ot[:])
                nc.sync.dma_start(taps["ot0"], ots[:])

        if taps is not None:
            for hp in range(4):
                nc.sync.dma_start(taps[f"yt{hp}"], yt_t[hp][:])

        # ---- output projection: out[q, n] = sum_hp yT_hp.T @ W_hp ----
        for qt in range(NQB):
            for nh in range(2):
                op_t = proj_pool.tile([128, 512], F32, tag="op")
                for hp in range(4):
                    nc.tensor.matmul(
                        op_t[:],
                        lhsT=yt_t[hp][:, qt * 128:(qt + 1) * 128],
                        rhs=w_t[hp][:, nh * 512:(nh + 1) * 512],
                        start=(hp == 0), stop=(hp == 3),
                    )
                ost = ost_pool.tile([128, 512], F32, tag="ost")
                nc.vector.tensor_copy(ost[:], op_t[:])
                nc.sync.dma_start(
                    out_d[qt * 128:(qt + 1) * 128, nh * 512:(nh + 1) * 512],
                    ost[:],
                )


def build_program(debug_taps=False):
    """Build + compile the SPMD program once.  Returns the Bacc object."""
    from concourse import bacc, tile
    import concourse.mybir as mybir

    BF16 = mybir.dt.bfloat16
    F32 = mybir.dt.float32

    nc = bacc.Bacc("TRN2", target_bir_lowering=False, debug=False, num_devices=8)
    qT_d = nc.dram_tensor("qT", [HL, 65, QL], BF16, kind="ExternalInput").ap()
    kT_d = nc.dram_tensor("kT", [HL, 65, KW], BF16, kind="ExternalInput").ap()
    vE_d = nc.dram_tensor("vE", [HL, 128, NKT * 128], BF16, kind="ExternalInput").ap()
    Wc_d = nc.dram_tensor("Wc", [4, 128, DIM], BF16, kind="ExternalInput").ap()
    tri_d = nc.dram_tensor("tri", [2, 128, 128], BF16, kind="ExternalInput").ap()
    out_d = nc.dram_tensor("out", [QL, DIM], F32, kind="ExternalOutput").ap()

    taps = None
    if debug_taps:
        taps = {
            "pt0": nc.dram_tensor("pt0", [128, 1536], BF16, kind="ExternalOutput").ap(),
            "rc0": nc.dram_tensor("rc0", [64, QL], F32, kind="ExternalOutput").ap(),
            "ot0": nc.dram_tensor("ot0", [128, QL], F32, kind="ExternalOutput").ap(),
        }
        for hp in range(4):
            taps[f"yt{hp}"] = nc.dram_tensor(
                f"yt{hp}", [128, QL], BF16, kind="ExternalOutput"
            ).ap()

    with tile.TileContext(nc) as tc:
        _emit(tc, qT_d, kT_d, vE_d, Wc_d, tri_d, out_d, taps=taps)
    nc.compile()
    return nc


def pack_inputs(q, k, v, W_proj):
    """Shard + lay out the full inputs for the 8 cores.  Returns in_maps."""
    q = np.asarray(q, dtype=np.float32)
    k = np.asarray(k, dtype=np.float32)
    v = np.asarray(v, dtype=np.float32)
    W = np.asarray(W_proj, dtype=np.float32)

    p_idx = np.arange(128)[:, None]
    i_idx = np.arange(128)[None, :]
    tri = np.stack([
        (p_idx > i_idx).astype(np.float32),
        (p_idx <= i_idx).astype(np.float32),
    ]).astype(BF16NP)

    in_maps = []
    for c in range(8):
        r, s = c // S, c % S
        hs = slice(r * HL, (r + 1) * HL)
        qs = slice(s * QL, (s + 1) * QL)

        qh = q[0, hs, qs, :]                      # (HL, QL, D)
        qT = np.empty((HL, 65, QL), dtype=np.float32)
        qT[:, :64, :] = qh.transpose(0, 2, 1)
        qT[:, 64, :] = 1.0

        j0 = s * QL - WINDOW
        idx = j0 + np.arange(KW)
        valid = idx >= 0
        kh = np.zeros((HL, KW, D), dtype=np.float32)
        vh = np.zeros((HL, KW, D), dtype=np.float32)
        kh[:, valid] = k[0, hs][:, idx[valid], :]
        vh[:, valid] = v[0, hs][:, idx[valid], :]

        kT = np.empty((HL, 65, KW), dtype=np.float32)
        kT[:, :64, :] = kh.transpose(0, 2, 1)
        kT[:, 64, :] = np.where(valid, 0.0, NEG)[None, :]

        vE = np.ones((HL, 128, NKT * 128), dtype=np.float32)
        for kt in range(NKT):
            vE[:, :, kt * 128 + 64:(kt + 1) * 128] = vh[:, kt * 128:(kt + 1) * 128, :]

        Wc = np.ascontiguousarray(
            W[r * 512:(r + 1) * 512, :].reshape(4, 128, DIM)
        )

        in_maps.append({
            "qT": qT.astype(BF16NP),
            "kT": kT.astype(BF16NP),
            "vE": vE.astype(BF16NP),
            "Wc": Wc.astype(BF16NP),
            "tri": tri,
        })
    return in_maps


def combine_outputs(results):
    """results[c]["out"] -> full (B, T, DIM) float32 output."""
    out = np.zeros((B, T, DIM), dtype=np.float32)
    for c in range(8):
        r, s = c // S, c % S
        out[0, s * QL:(s + 1) * QL, :] += results[c]["out"]
    return out


_PROGRAM = None


def _get_program():
    global _PROGRAM
    if _PROGRAM is None:
        _PROGRAM = build_program()
    return _PROGRAM


def kernel(q, k, v, W_proj):
    from concourse.bass_utils import run_bass_kernel_spmd

    nc = _get_program()
    in_maps = pack_inputs(q, k, v, W_proj)
    res = run_bass_kernel_spmd(nc, in_maps, list(range(8)))
    return combine_outputs(res.results)


if __name__ == "__main__":
    # smoke test with random data
    rng = np.random.default_rng(0)
    q = rng.standard_normal((B, H, T, D), dtype=np.float32)
    k = rng.standard_normal((B, H, T, D), dtype=np.float32)
    v = rng.standard_normal((B, H, T, D), dtype=np.float32)
    W = rng.standard_normal((DIM, DIM), dtype=np.float32) / math.sqrt(DIM)
    out = kernel(q, k, v, W)
    print(out.shape, out.dtype, np.abs(out).mean())
